# revision 7
# baseline (speedup 1.0000x reference)
"""Trainium2 Bass kernel for nn_CCSequenceModel (2-layer GRU encoder + autoregressive
2-layer GRU decoder with gated output head).

Strategy: pure data parallel over 8 NeuronCores (batch 8192 -> 1024/core).
Gate/hidden dim on partitions, batch on the free dim, two 512-sample halves
(PSUM bank limit). SBUF bf16, PSUM fp32.

Encoder (gate-packed across layers): the two staggered cells (L0 consuming
x_k,h0_{k-1}; L1 consuming h0_{k-1},h1_{k-2}) are packed per GATE on the
partition axis: four psum banks per half, Z=[z0|z1], R=[r0|r1], IN=[in0|in1],
HN=[hn0|hn1] (cell0 partitions 0:64, cell1 64:128). Each bank is filled by two
64-col-stationary matmuls to disjoint col-groups (concurrent on the PE array).
One sigmoid per gate-bank (per-partition bias [b_c0|b_c1]), ONE tanh / STT /
acc / blend chain covers both cells. Per tick per half:
  Z,R,IN,HN <- 8 matmuls;  zt=sig(Z+bz); rt=sig(R+br)
  rhn = (HN + bhh_n) * rt           (DVE STT, 128p)
  IN += I128 @ rhn                  (PE identity accumulation)
  n = tanh(IN + bih_n)              (ACT, 128p)
  u = S1 - n; v = zt*u; S1' = n + v (DVE, 128p)  -> [h0_k | h1_{k-1}]
  S0'[0:64] = S1'[0:64]             (DVE copy for L0's next stream)
Decoder: per-cell [z|r]/[in|hn] banks as before, but dec1's matmuls are split
into early rows (Whh @ h1_{t-1}, issued before dec0's chain) and late rows
(Wih @ h0d_t, accumulating, reading dec0's output tile directly) - the h0d
copies are gone. cv feedback via heads + STT(is_gt)*mult as before.
x is pre-transposed host-side to (L, 6, B_core) bf16; cv staged to DRAM (T,
B_core) and transposed on host.
"""
import sys
import numpy as np

for _p in ('/opt/trn_rl_repo', '/root/.axon_site/_ro/trn_rl_repo'):
    if _p not in sys.path:
        sys.path.insert(0, _p)

import ml_dtypes
import concourse.bass as bass
import concourse.tile as tile
from concourse import bacc, mybir
from concourse.bass_utils import run_bass_kernel_spmd

BF16 = mybir.dt.bfloat16
F32 = mybir.dt.float32
NPBF = ml_dtypes.bfloat16
ALU = mybir.AluOpType
ACTF = mybir.ActivationFunctionType

H = 64
NIN = 6
NCORES = 8
T_OUT = 180

_BUILD_CACHE = {}

LO, HI = slice(0, 64), slice(64, 128)


# ------------------------------------------------------------------ host prep
def _pack_cell(Wih, Whh, bih, bhh, in_rows, h_rows, blend_lo, K):
    """Decoder cell packing (unchanged from the per-cell scheme):
    W1 (K,128) = [z|r] with z on the blend side, W2 (K,128) = [in|hn]."""
    Wih = np.asarray(Wih, np.float32)
    Whh = np.asarray(Whh, np.float32)
    bih = np.asarray(bih, np.float32)
    bhh = np.asarray(bhh, np.float32)
    W1 = np.zeros((K, 128), np.float32)
    W2 = np.zeros((K, 128), np.float32)
    b1 = np.zeros(128, np.float32)
    b2 = np.zeros(128, np.float32)
    r, z, n = slice(0, 64), slice(64, 128), slice(128, 192)
    lo, hi = slice(0, 64), slice(64, 128)
    zc, rc = (lo, hi) if blend_lo else (hi, lo)
    inc, hnc = (lo, hi) if blend_lo else (hi, lo)
    W1[in_rows, zc] = Wih[z].T
    W1[h_rows, zc] = Whh[z].T
    W1[in_rows, rc] = Wih[r].T
    W1[h_rows, rc] = Whh[r].T
    W2[in_rows, inc] = Wih[n].T
    W2[h_rows, hnc] = Whh[n].T
    b1[zc] = bih[z] + bhh[z]
    b1[rc] = bih[r] + bhh[r]
    b2[inc] = bih[n]
    b2[hnc] = bhh[n]
    return W1, W2, b1, b2


def _prep(inputs, BC):
    g = lambda k: np.asarray(inputs[k], np.float32)
    out = {}
    r_, z_, n_ = slice(0, 64), slice(64, 128), slice(128, 192)

    # ---- encoder, gate-packed across cells
    W0ih, W0hh = g('enc0_Wih'), g('enc0_Whh')
    W1ih, W1hh = g('enc1_Wih'), g('enc1_Whh')
    b0ih, b0hh = g('enc0_bih'), g('enc0_bhh')
    b1ih, b1hh = g('enc1_bih'), g('enc1_bhh')

    def gate0(sl):  # cell0 stationary [70,64]: rows 0:64 h0 (Whh), 64:70 x (Wih)
        W = np.zeros((70, 64), np.float32)
        W[0:64] = W0hh[sl].T
        W[64:70] = W0ih[sl].T
        return W

    def gate1(sl):  # cell1 stationary [128,64]: rows 0:64 h0 (Wih), 64:128 h1 (Whh)
        W = np.zeros((128, 64), np.float32)
        W[0:64] = W1ih[sl].T
        W[64:128] = W1hh[sl].T
        return W

    out['wz0'] = gate0(z_).astype(NPBF)
    out['wr0'] = gate0(r_).astype(NPBF)
    win0 = np.zeros((70, 64), np.float32)
    win0[64:70] = W0ih[n_].T
    out['win0'] = win0.astype(NPBF)
    out['whn0'] = np.ascontiguousarray(W0hh[n_].T).astype(NPBF)   # (64,64)
    out['wz1'] = gate1(z_).astype(NPBF)
    out['wr1'] = gate1(r_).astype(NPBF)
    winhn1 = np.zeros((128, 64), np.float32)   # rows 0:64 = Wih_n1 (reads h0),
    winhn1[0:64] = W1ih[n_].T                  # rows 64:128 = Whh_n1 (reads h1)
    winhn1[64:128] = W1hh[n_].T
    out['winhn1'] = winhn1.astype(NPBF)
    out['ident'] = np.eye(128, dtype=np.float32).astype(NPBF)
    identb = np.zeros((128, 64), np.float32)   # both row-halves hold I64
    identb[0:64] = np.eye(64)                  # (decoder cross-block acc)
    identb[64:128] = np.eye(64)
    out['identb'] = identb.astype(NPBF)

    # ---- decoder (per-cell packing)
    W1, W2, b1, b2 = _pack_cell(g('dec0_Wih'), g('dec0_Whh'), g('dec0_bih'),
                                g('dec0_bhh'), slice(64, 65), slice(0, 64),
                                True, 65)
    out['w1_d0'], out['w2_d0'] = W1.astype(NPBF), W2.astype(NPBF)
    bd0_1, bd0_2 = b1, b2
    W1, W2, b1, b2 = _pack_cell(g('dec1_Wih'), g('dec1_Whh'), g('dec1_bih'),
                                g('dec1_bhh'), slice(0, 64), slice(64, 128),
                                False, 128)
    out['w1_d1'], out['w2_d1'] = W1.astype(NPBF), W2.astype(NPBF)
    bd1_1, bd1_2 = b1, b2

    won = np.zeros((64, 1), np.float32)
    won[:, 0] = g('on_w')[0]
    wcv = np.zeros((64, 1), np.float32)
    wcv[:, 0] = g('cv_w')[0]
    out['w_on'], out['w_cv'] = won.astype(NPBF), wcv.astype(NPBF)

    # bias pack (128, 12): 0=bz 1=br 2=bin 3=bhn (encoder, [c0|c1]);
    # 4..7 decoder (d0_1, d0_2, d1_1, d1_2); 8=-on_b@64, 9=cv_b@64
    bias = np.zeros((128, 12), np.float32)
    bias[0:64, 0] = b0ih[z_] + b0hh[z_]
    bias[64:128, 0] = b1ih[z_] + b1hh[z_]
    bias[0:64, 1] = b0ih[r_] + b0hh[r_]
    bias[64:128, 1] = b1ih[r_] + b1hh[r_]
    bias[0:64, 2] = b0ih[n_]
    bias[64:128, 2] = b1ih[n_]
    bias[0:64, 3] = b0hh[n_]
    bias[64:128, 3] = b1hh[n_]
    for j, b in enumerate([bd0_1, bd0_2, bd1_1, bd1_2]):
        bias[:, 4 + j] = b
    bias[64, 8] = -float(g('on_b')[0])
    bias[64, 9] = float(g('cv_b')[0])
    out['biases'] = bias
    return out


# ------------------------------------------------------------------ device build
def _build(L, T, BC):
    M = BC // 2
    nc = bacc.Bacc("TRN2", target_bir_lowering=False, debug=False,
                   num_devices=NCORES)
    dram = {}
    for name, shape, dt in [
        ('xT', [L, NIN, BC], BF16),
        ('wz0', [70, 64], BF16), ('wr0', [70, 64], BF16),
        ('win0', [70, 64], BF16), ('whn0', [64, 64], BF16),
        ('wz1', [128, 64], BF16), ('wr1', [128, 64], BF16),
        ('winhn1', [128, 64], BF16), ('ident', [128, 128], BF16),
        ('identb', [128, 64], BF16),
        ('w1_d0', [65, 128], BF16), ('w2_d0', [65, 128], BF16),
        ('w1_d1', [128, 128], BF16), ('w2_d1', [128, 128], BF16),
        ('w_on', [64, 1], BF16), ('w_cv', [64, 1], BF16),
        ('biases', [128, 12], F32),
    ]:
        dram[name] = nc.dram_tensor(name, shape, dt, kind="ExternalInput").ap()
    stg = nc.dram_tensor("stg", [T, BC], BF16, kind="ExternalOutput").ap()

    with tile.TileContext(nc) as tc:
        const = tc.alloc_tile_pool(name="const", bufs=1)
        work = tc.alloc_tile_pool(name="work", bufs=3)

        cw = {}
        for name in ['wz0', 'wr0', 'win0', 'whn0', 'wz1', 'wr1', 'winhn1',
                     'ident', 'identb', 'w1_d0', 'w2_d0', 'w1_d1', 'w2_d1']:
            t_ = const.tile(list(dram[name].shape), BF16, name=f"c_{name}")
            nc.sync.dma_start(out=t_, in_=dram[name])
            cw[name] = t_
        whead = const.tile([128, 2], BF16, name="c_whead")
        nc.sync.dma_start(out=whead[64:128, 0:1], in_=dram['w_on'])
        nc.sync.dma_start(out=whead[64:128, 1:2], in_=dram['w_cv'])
        bias = const.tile([128, 12], F32, name="c_bias")
        nc.sync.dma_start(out=bias, in_=dram['biases'])

        bcol = lambda j: bias[:, j:j + 1]

        # persistent stream tiles
        s0 = [const.tile([70, BC], BF16, name=f"s0_{i}") for i in range(3)]
        s1 = [const.tile([128, BC], BF16, name=f"s1_{i}") for i in range(2)]
        sd0 = [const.tile([65, BC], BF16, name=f"sd0_{i}") for i in range(2)]
        sd1 = [const.tile([128, BC], BF16, name=f"sd1_{i}") for i in range(2)]

        nc.vector.memset(s0[0][LO, :], 0.0)
        nc.vector.memset(s1[0][:, :], 0.0)
        nc.vector.memset(s1[1][HI, :], 0.0)
        nc.sync.dma_start(out=s0[0][64:70, :], in_=dram['xT'][0])

        def hs(ap, h):
            return ap[:, h * M:(h + 1) * M]

        # ================= encoder =================
        eps = tc.alloc_tile_pool(name="eps", bufs=1, space="PSUM")
        for k in range(L + 1):
            S0, S0n = s0[k % 3], s0[(k + 1) % 3]
            S1, S1n = s1[k % 2], s1[(k + 1) % 2]
            if 1 <= k + 1 < L:
                nc.sync.dma_start(out=S0n[64:70, :], in_=dram['xT'][k + 1])
            has0, has1 = k < L, k >= 1
            if has0 and has1:
                part = slice(0, 128)
            else:
                part = LO if has0 else HI

            pz = [eps.tile([128, M], F32, name=f"pz{h}_{k}", tag=f"z{h}")
                  for h in range(2)]
            pr = [eps.tile([128, M], F32, name=f"pr{h}_{k}", tag=f"r{h}")
                  for h in range(2)]
            pin = [eps.tile([128, M], F32, name=f"pin{h}_{k}", tag=f"i{h}")
                   for h in range(2)]
            phn = [eps.tile([128, M], F32, name=f"phn{h}_{k}", tag=f"n{h}")
                   for h in range(2)]
            zt = work.tile([128, BC], BF16, name=f"zt_{k}", tag="zt")
            rt = work.tile([128, BC], BF16, name=f"rt_{k}", tag="rt")
            rhn = work.tile([128, BC], BF16, name=f"rhn_{k}", tag="rhn")
            n_t = work.tile([128, BC], BF16, name=f"n_{k}", tag="n")
            u_t = work.tile([128, BC], BF16, name=f"u_{k}", tag="u")
            v_t = work.tile([128, BC], BF16, name=f"v_{k}", tag="v")

            # --- matmul stage (per half: 8 gate MMs as 4 col-concurrent pairs)
            for h in range(2):
                if has0:
                    nc.tensor.matmul(pz[h][LO, :], cw['wz0'], hs(S0, h),
                                     start=True, stop=True, tile_position=(0, 0))
                if has1:
                    nc.tensor.matmul(pz[h][HI, :], cw['wz1'], hs(S1, h),
                                     start=True, stop=True, tile_position=(0, 64))
            for h in range(2):
                if has0:
                    nc.tensor.matmul(pr[h][LO, :], cw['wr0'], hs(S0, h),
                                     start=True, stop=True, tile_position=(0, 0))
                if has1:
                    nc.tensor.matmul(pr[h][HI, :], cw['wr1'], hs(S1, h),
                                     start=True, stop=True, tile_position=(0, 64))
            for h in range(2):
                if has0:
                    nc.tensor.matmul(pin[h][LO, :], cw['win0'], hs(S0, h),
                                     start=True, stop=False,
                                     tile_position=(0, 0))
                if has1:
                    nc.tensor.matmul(pin[h][HI, :], cw['winhn1'][LO, :],
                                     hs(S1, h)[LO, :], start=True, stop=False,
                                     tile_position=(0, 64))
            for h in range(2):
                if has0:
                    nc.tensor.matmul(phn[h][LO, :], cw['whn0'],
                                     hs(S0, h)[LO, :], start=True, stop=True,
                                     tile_position=(0, 0))
                if has1:
                    nc.tensor.matmul(phn[h][HI, :], cw['winhn1'][HI, :],
                                     hs(S1, h)[HI, :], start=True, stop=True,
                                     tile_position=(64, 64))
            # --- activations / blend (per half, both cells at once)
            for h in range(2):
                nc.scalar.activation(out=hs(zt, h)[part, :],
                                     in_=pz[h][part, :], func=ACTF.Sigmoid,
                                     bias=bcol(0)[part, :], scale=1.0)
            for h in range(2):
                nc.scalar.activation(out=hs(rt, h)[part, :],
                                     in_=pr[h][part, :], func=ACTF.Sigmoid,
                                     bias=bcol(1)[part, :], scale=1.0)
            for h in range(2):
                nc.vector.scalar_tensor_tensor(
                    out=hs(rhn, h)[part, :], in0=phn[h][part, :],
                    scalar=bcol(3)[part, :], in1=hs(rt, h)[part, :],
                    op0=ALU.add, op1=ALU.mult)
            for h in range(2):
                if has0 and has1:
                    nc.tensor.matmul(pin[h][:, :], cw['ident'],
                                     hs(rhn, h)[:, :], start=False, stop=True,
                                     tile_position=(0, 0))
                elif has0:
                    nc.tensor.matmul(pin[h][LO, :], cw['ident'][LO, LO],
                                     hs(rhn, h)[LO, :], start=False, stop=True,
                                     tile_position=(0, 0))
                else:
                    nc.tensor.matmul(pin[h][HI, :], cw['ident'][HI, HI],
                                     hs(rhn, h)[HI, :], start=False, stop=True,
                                     tile_position=(64, 64))
            for h in range(2):
                nc.scalar.activation(out=hs(n_t, h)[part, :],
                                     in_=pin[h][part, :], func=ACTF.Tanh,
                                     bias=bcol(2)[part, :], scale=1.0)
            for h in range(2):
                nc.vector.tensor_tensor(out=hs(u_t, h)[part, :],
                                        in0=hs(S1, h)[part, :],
                                        in1=hs(n_t, h)[part, :],
                                        op=ALU.subtract)
            for h in range(2):
                nc.vector.tensor_tensor(out=hs(v_t, h)[part, :],
                                        in0=hs(zt, h)[part, :],
                                        in1=hs(u_t, h)[part, :], op=ALU.mult)
            for h in range(2):
                nc.vector.tensor_tensor(out=hs(S1n, h)[part, :],
                                        in0=hs(n_t, h)[part, :],
                                        in1=hs(v_t, h)[part, :], op=ALU.add)
            if k < L - 1:
                for h in range(2):
                    nc.vector.tensor_copy(hs(S0n, h)[LO, :],
                                          hs(S1n, h)[LO, :])

        # ================= transition =================
        nc.vector.tensor_copy(sd0[0][LO, :], s1[L % 2][LO, :])
        nc.vector.tensor_copy(sd1[0][HI, :], s1[(L + 1) % 2][HI, :])
        nc.vector.memset(sd0[0][64:65, :], 0.0)
        eps.release()

        # ================= decoder =================
        dps = tc.alloc_tile_pool(name="dps", bufs=1, space="PSUM")
        for t in range(T):
            D0, D0n = sd0[t % 2], sd0[(t + 1) % 2]
            D1, D1n = sd1[t % 2], sd1[(t + 1) % 2]
            n_t = work.tile([128, BC], BF16, name=f"nd_{t}", tag="n")
            u_t = work.tile([128, BC], BF16, name=f"ud_{t}", tag="u")
            v_t = work.tile([128, BC], BF16, name=f"vd_{t}", tag="v")
            rz0 = work.tile([128, BC], BF16, name=f"rz0_{t}", tag="zt")
            rz1 = work.tile([128, BC], BF16, name=f"rz1_{t}", tag="rt")
            rhn = work.tile([128, BC], BF16, name=f"rhnd_{t}", tag="rhn")
            cvsb = work.tile([66, BC], BF16, name=f"cvsb_{t}", tag="cvsb")
            p1_0 = [dps.tile([128, M], F32, name=f"p10{h}_{t}", tag=f"d0a{h}")
                    for h in range(2)]
            p2_0 = [dps.tile([128, M], F32, name=f"p20{h}_{t}", tag=f"d0b{h}")
                    for h in range(2)]
            p1_1 = [dps.tile([128, M], F32, name=f"p11{h}_{t}", tag=f"d1a{h}")
                    for h in range(2)]
            p2_1 = [dps.tile([128, M], F32, name=f"p21{h}_{t}", tag=f"d1b{h}")
                    for h in range(2)]
            pon = [dps.tile([65, M], F32, name=f"pon{h}_{t}", tag=f"d0a{h}")
                   for h in range(2)]
            pcv = [dps.tile([65, M], F32, name=f"pcv{h}_{t}", tag=f"d0b{h}")
                   for h in range(2)]

            # dec1 EARLY matmuls: Whh part, reads h1_{t-1} from D1[HI]
            for h in range(2):
                nc.tensor.matmul(p1_1[h], cw['w1_d1'][HI, :],
                                 hs(D1, h)[HI, :], start=True, stop=False,
                                 tile_position=(64, 0))
                nc.tensor.matmul(p2_1[h], cw['w2_d1'][HI, :],
                                 hs(D1, h)[HI, :], start=True, stop=False,
                                 tile_position=(64, 0))
            # ---- dec0 (stream [h0d; cv] = D0)
            for h in range(2):
                nc.tensor.matmul(p1_0[h], cw['w1_d0'], hs(D0, h),
                                 start=True, stop=True, tile_position=(0, 0))
                nc.tensor.matmul(p2_0[h], cw['w2_d0'], hs(D0, h),
                                 start=True, stop=True, tile_position=(0, 0))
            for h in range(2):
                nc.scalar.activation(out=hs(rz0, h), in_=p1_0[h],
                                     func=ACTF.Sigmoid, bias=bcol(4),
                                     scale=1.0)
            for h in range(2):
                nc.vector.scalar_tensor_tensor(
                    out=hs(rhn, h)[HI, :], in0=p2_0[h][HI, :],
                    scalar=bcol(5)[HI, :], in1=hs(rz0, h)[HI, :],
                    op0=ALU.add, op1=ALU.mult)
            for h in range(2):
                nc.tensor.matmul(p2_0[h][LO, :], cw['identb'][HI, :],
                                 hs(rhn, h)[HI, :], start=False, stop=True,
                                 tile_position=(64, 0))
            for h in range(2):
                nc.scalar.activation(out=hs(n_t, h)[LO, :],
                                     in_=p2_0[h][LO, :], func=ACTF.Tanh,
                                     bias=bcol(5)[LO, :], scale=1.0)
            for h in range(2):
                nc.vector.tensor_tensor(out=hs(u_t, h)[LO, :],
                                        in0=hs(D0, h)[LO, :],
                                        in1=hs(n_t, h)[LO, :],
                                        op=ALU.subtract)
            for h in range(2):
                nc.vector.tensor_tensor(out=hs(v_t, h)[LO, :],
                                        in0=hs(rz0, h)[LO, :],
                                        in1=hs(u_t, h)[LO, :], op=ALU.mult)
            for h in range(2):
                nc.vector.tensor_tensor(out=hs(D0n, h)[LO, :],
                                        in0=hs(n_t, h)[LO, :],
                                        in1=hs(v_t, h)[LO, :], op=ALU.add)
            # dec1 LATE matmuls: Wih part, reads h0d_t from D0n[LO]
            for h in range(2):
                nc.tensor.matmul(p1_1[h], cw['w1_d1'][LO, :],
                                 hs(D0n, h)[LO, :], start=False, stop=True,
                                 tile_position=(0, 0))
                nc.tensor.matmul(p2_1[h], cw['w2_d1'][LO, :],
                                 hs(D0n, h)[LO, :], start=False, stop=True,
                                 tile_position=(0, 0))
            for h in range(2):
                nc.scalar.activation(out=hs(rz1, h), in_=p1_1[h],
                                     func=ACTF.Sigmoid, bias=bcol(6),
                                     scale=1.0)
            for h in range(2):
                nc.vector.scalar_tensor_tensor(
                    out=hs(rhn, h)[LO, :], in0=p2_1[h][LO, :],
                    scalar=bcol(7)[LO, :], in1=hs(rz1, h)[LO, :],
                    op0=ALU.add, op1=ALU.mult)
            for h in range(2):
                nc.tensor.matmul(p2_1[h][HI, :], cw['identb'][LO, :],
                                 hs(rhn, h)[LO, :], start=False, stop=True,
                                 tile_position=(0, 64))
            for h in range(2):
                nc.scalar.activation(out=hs(n_t, h)[HI, :],
                                     in_=p2_1[h][HI, :], func=ACTF.Tanh,
                                     bias=bcol(7)[HI, :], scale=1.0)
            for h in range(2):
                nc.vector.tensor_tensor(out=hs(u_t, h)[HI, :],
                                        in0=hs(D1, h)[HI, :],
                                        in1=hs(n_t, h)[HI, :],
                                        op=ALU.subtract)
            for h in range(2):
                nc.vector.tensor_tensor(out=hs(v_t, h)[HI, :],
                                        in0=hs(rz1, h)[HI, :],
                                        in1=hs(u_t, h)[HI, :], op=ALU.mult)
            for h in range(2):
                nc.vector.tensor_tensor(out=hs(D1n, h)[HI, :],
                                        in0=hs(n_t, h)[HI, :],
                                        in1=hs(v_t, h)[HI, :], op=ALU.add)
            # heads on h1' (= D1n[HI])
            for h in range(2):
                nc.tensor.matmul(pon[h][64:65, :], whead[64:128, 0:1],
                                 hs(D1n, h)[HI, :], start=True, stop=True,
                                 tile_position=(64, 64))
                nc.tensor.matmul(pcv[h][64:65, :], whead[64:128, 1:2],
                                 hs(D1n, h)[HI, :], start=True, stop=True,
                                 tile_position=(64, 64))
            for h in range(2):
                nc.scalar.activation(out=hs(cvsb, h)[64:65, :],
                                     in_=pcv[h][64:65, :], func=ACTF.Identity,
                                     bias=bias[64:65, 9:10], scale=1.0)
            for h in range(2):
                nc.vector.scalar_tensor_tensor(
                    out=hs(D0n, h)[64:65, :], in0=pon[h][64:65, :],
                    scalar=bias[64:65, 8:9], in1=hs(cvsb, h)[64:65, :],
                    op0=ALU.is_gt, op1=ALU.mult)
            nc.gpsimd.dma_start(out=stg[t:t + 1, :], in_=D0n[64:65, :])

        dps.release()
        work.release()
        const.release()

    nc.compile()
    return nc


def _get_nc(L, T, BC):
    key = (L, T, BC)
    if key not in _BUILD_CACHE:
        _BUILD_CACHE[key] = _build(L, T, BC)
    return _BUILD_CACHE[key]


# ------------------------------------------------------------------ entry point
def kernel(**inputs):
    x = np.asarray(inputs['x'])
    B, L, _ = x.shape
    T = T_OUT
    BC = B // NCORES
    nc = _get_nc(L, T, BC)

    packed = _prep(inputs, BC)
    in_maps = []
    for c in range(NCORES):
        xs = x[c * BC:(c + 1) * BC].astype(np.float32)      # (BC, L, 6)
        xT = np.ascontiguousarray(xs.transpose(1, 2, 0)).astype(NPBF)
        m = dict(packed)
        m['xT'] = xT
        in_maps.append(m)

    res = run_bass_kernel_spmd(nc, in_maps, core_ids=list(range(NCORES)))
    out = np.empty((B, T, 1), np.float32)
    for c in range(NCORES):
        stg = np.asarray(res.results[c]['stg'], np.float32)  # (T, BC)
        out[c * BC:(c + 1) * BC, :, 0] = stg.T
    return out


# revision 17
# speedup vs baseline: 1.1476x; 1.1476x over previous
"""Trainium2 Bass kernel for nn_CCSequenceModel (2-layer GRU encoder + autoregressive
2-layer GRU decoder with gated output head).

Strategy: pure data parallel over 8 NeuronCores (batch 8192 -> 1024/core).
Gate/hidden dim on partitions, batch on the free dim, two 512-sample halves
(PSUM bank limit), processed as two software-pipelined chains offset by half a
tick so one half's matmuls run under the other half's elementwise phase.

Encoder (merged stream + gate-packed banks): state tile S = [h0 | h1] (128p);
x rides a separate 7-partition tile (6 inputs + a ones row). The two staggered
cells (L0 consuming x_k,h0_{k-1}; L1 consuming h0_{k-1},h1_{k-2}) are packed
per GATE on the partition axis; per half, four psum banks: ZR=[z0|z1 ; r0|r1]
(2 banks), IN=[in0|in1], HN=[hn0|hn1]. Each bank is filled by one 128-col
matmul streaming S plus one 7-row matmul streaming the x tile whose ones row
carries ALL biases (both cells), so:
  szr = sigmoid(ZR)                 (ACT, one FD1024 op, bias-free)
  rhn = (HN + bhh_n) * szr[r]       (DVE STT, 128p)
  IN += I128 @ rhn                  (PE identity accumulation)
  n = tanh(IN)                      (ACT, bias baked)
  u = S - n; v = szr[z]*u; S' = n+v (DVE)   -> [h0_k | h1_{k-1}]
Decoder: per-cell [z|r]/[in|hn] banks; dec1's matmuls split into early rows
(Whh @ h1_{t-1}, issued before dec0's chain) and late rows (Wih @ h0d_t,
accumulating, reading dec0's output tile directly) - no h0d copies. cv
feedback via heads + STT(is_gt)*mult. Emission is per-(tick, half) blocks for
the software pipeline. x pre-transposed host-side to (L, 7, B_core) bf16; cv
staged to DRAM (T, B_core), host transposes.
"""
import sys
import numpy as np

for _p in ('/opt/trn_rl_repo', '/root/.axon_site/_ro/trn_rl_repo'):
    if _p not in sys.path:
        sys.path.insert(0, _p)

import ml_dtypes
import concourse.bass as bass
import concourse.tile as tile
from concourse import bacc, mybir
from concourse.bass_utils import run_bass_kernel_spmd

BF16 = mybir.dt.bfloat16
F32 = mybir.dt.float32
NPBF = ml_dtypes.bfloat16
ALU = mybir.AluOpType
ACTF = mybir.ActivationFunctionType

H = 64
NIN = 6
NCORES = 8
T_OUT = 180

_BUILD_CACHE = {}

LO, HI = slice(0, 64), slice(64, 128)


# ------------------------------------------------------------------ host prep
def _pack_cell(Wih, Whh, bih, bhh, in_rows, h_rows, blend_lo, K):
    """Decoder cell packing: W1 (K,128) = [z|r] with z on the blend side,
    W2 (K,128) = [in|hn]."""
    Wih = np.asarray(Wih, np.float32)
    Whh = np.asarray(Whh, np.float32)
    bih = np.asarray(bih, np.float32)
    bhh = np.asarray(bhh, np.float32)
    W1 = np.zeros((K, 128), np.float32)
    W2 = np.zeros((K, 128), np.float32)
    b1 = np.zeros(128, np.float32)
    b2 = np.zeros(128, np.float32)
    r, z, n = slice(0, 64), slice(64, 128), slice(128, 192)
    lo, hi = slice(0, 64), slice(64, 128)
    zc, rc = (lo, hi) if blend_lo else (hi, lo)
    inc, hnc = (lo, hi) if blend_lo else (hi, lo)
    W1[in_rows, zc] = Wih[z].T
    W1[h_rows, zc] = Whh[z].T
    W1[in_rows, rc] = Wih[r].T
    W1[h_rows, rc] = Whh[r].T
    W2[in_rows, inc] = Wih[n].T
    W2[h_rows, hnc] = Whh[n].T
    b1[zc] = bih[z] + bhh[z]
    b1[rc] = bih[r] + bhh[r]
    b2[inc] = bih[n]
    b2[hnc] = bhh[n]
    return W1, W2, b1, b2


def _prep(inputs, BC):
    g = lambda k: np.asarray(inputs[k], np.float32)
    out = {}
    r_, z_, n_ = slice(0, 64), slice(64, 128), slice(128, 192)

    # ---- encoder, gate-packed across cells (64-col stationaries, paired)
    W0ih, W0hh = g('enc0_Wih'), g('enc0_Whh')
    W1ih, W1hh = g('enc1_Wih'), g('enc1_Whh')
    b0ih, b0hh = g('enc0_bih'), g('enc0_bhh')
    b1ih, b1hh = g('enc1_bih'), g('enc1_bhh')

    def gate0(sl):  # cell0 stationary [70,64]: rows 0:64 h0 (Whh), 64:70 x
        W = np.zeros((70, 64), np.float32)
        W[0:64] = W0hh[sl].T
        W[64:70] = W0ih[sl].T
        return W.astype(NPBF)

    def gate1(sl):  # cell1 stationary [128,64]: rows 0:64 h0 (Wih), 64:128 h1
        W = np.zeros((128, 64), np.float32)
        W[0:64] = W1ih[sl].T
        W[64:128] = W1hh[sl].T
        return W.astype(NPBF)

    out['wz0'] = gate0(z_)
    out['wr0'] = gate0(r_)
    win0 = np.zeros((70, 64), np.float32)
    win0[64:70] = W0ih[n_].T
    out['win0'] = win0.astype(NPBF)
    out['whn0'] = np.ascontiguousarray(W0hh[n_].T).astype(NPBF)   # (64,64)
    out['wz1'] = gate1(z_)
    out['wr1'] = gate1(r_)
    winhn1 = np.zeros((128, 64), np.float32)   # rows 0:64 = Wih_n1 (reads h0),
    winhn1[0:64] = W1ih[n_].T                  # rows 64:128 = Whh_n1 (reads h1)
    winhn1[64:128] = W1hh[n_].T
    out['winhn1'] = winhn1.astype(NPBF)
    out['ident'] = np.eye(128, dtype=np.float32).astype(NPBF)
    identb = np.zeros((128, 64), np.float32)   # both row-halves hold I64
    identb[0:64] = np.eye(64)                  # (decoder cross-block acc)
    identb[64:128] = np.eye(64)
    out['identb'] = identb.astype(NPBF)

    # ---- decoder (per-cell packing)
    W1, W2, b1, b2 = _pack_cell(g('dec0_Wih'), g('dec0_Whh'), g('dec0_bih'),
                                g('dec0_bhh'), slice(64, 65), slice(0, 64),
                                True, 65)
    out['w1_d0'], out['w2_d0'] = W1.astype(NPBF), W2.astype(NPBF)
    bd0_1, bd0_2 = b1, b2
    W1, W2, b1, b2 = _pack_cell(g('dec1_Wih'), g('dec1_Whh'), g('dec1_bih'),
                                g('dec1_bhh'), slice(0, 64), slice(64, 128),
                                False, 128)
    out['w1_d1'], out['w2_d1'] = W1.astype(NPBF), W2.astype(NPBF)
    bd1_1, bd1_2 = b1, b2

    won = np.zeros((64, 1), np.float32)
    won[:, 0] = g('on_w')[0]
    wcv = np.zeros((64, 1), np.float32)
    wcv[:, 0] = g('cv_w')[0]
    out['w_on'], out['w_cv'] = won.astype(NPBF), wcv.astype(NPBF)

    # bias pack (128, 12): 0=bz 1=br 2=bin 3=bhh_n (encoder, [c0|c1]);
    # 4..7 decoder; 8=-on_b@64, 9=cv_b@64
    bias = np.zeros((128, 12), np.float32)
    bias[0:64, 0] = b0ih[z_] + b0hh[z_]
    bias[64:128, 0] = b1ih[z_] + b1hh[z_]
    bias[0:64, 1] = b0ih[r_] + b0hh[r_]
    bias[64:128, 1] = b1ih[r_] + b1hh[r_]
    bias[0:64, 2] = b0ih[n_]
    bias[64:128, 2] = b1ih[n_]
    bias[0:64, 3] = b0hh[n_]
    bias[64:128, 3] = b1hh[n_]
    for j, b in enumerate([bd0_1, bd0_2, bd1_1, bd1_2]):
        bias[:, 4 + j] = b
    bias[64, 8] = -float(g('on_b')[0])
    bias[64, 9] = float(g('cv_b')[0])
    out['biases'] = bias
    return out


# ------------------------------------------------------------------ device build
def _build(L, T, BC):
    M = BC // 2
    nc = bacc.Bacc("TRN2", target_bir_lowering=False, debug=False,
                   num_devices=NCORES)
    dram = {}
    for name, shape, dt in [
        ('xT', [L, NIN, BC], BF16),
        ('wz0', [70, 64], BF16), ('wr0', [70, 64], BF16),
        ('win0', [70, 64], BF16), ('whn0', [64, 64], BF16),
        ('wz1', [128, 64], BF16), ('wr1', [128, 64], BF16),
        ('winhn1', [128, 64], BF16), ('ident', [128, 128], BF16),
        ('identb', [128, 64], BF16),
        ('w1_d0', [65, 128], BF16), ('w2_d0', [65, 128], BF16),
        ('w1_d1', [128, 128], BF16), ('w2_d1', [128, 128], BF16),
        ('w_on', [64, 1], BF16), ('w_cv', [64, 1], BF16),
        ('biases', [128, 12], F32),
    ]:
        dram[name] = nc.dram_tensor(name, shape, dt, kind="ExternalInput").ap()
    stg = nc.dram_tensor("stg", [T, BC], BF16, kind="ExternalOutput").ap()

    with tile.TileContext(nc) as tc:
        const = tc.alloc_tile_pool(name="const", bufs=1)
        work = tc.alloc_tile_pool(name="work", bufs=3)

        cw = {}
        for name in ['wz0', 'wr0', 'win0', 'whn0', 'wz1', 'wr1', 'winhn1',
                     'ident', 'identb', 'w1_d0', 'w2_d0', 'w1_d1', 'w2_d1']:
            t_ = const.tile(list(dram[name].shape), BF16, name=f"c_{name}")
            nc.sync.dma_start(out=t_, in_=dram[name])
            cw[name] = t_
        whead = const.tile([128, 2], BF16, name="c_whead")
        nc.sync.dma_start(out=whead[64:128, 0:1], in_=dram['w_on'])
        nc.sync.dma_start(out=whead[64:128, 1:2], in_=dram['w_cv'])
        bias = const.tile([128, 12], F32, name="c_bias")
        nc.sync.dma_start(out=bias, in_=dram['biases'])

        bcol = lambda j: bias[:, j:j + 1]

        # persistent stream tiles: s0 = [h0; x] (70p), s1 = [h0 | h1] (128p)
        s0 = [const.tile([70, BC], BF16, name=f"s0_{i}") for i in range(3)]
        s1 = [const.tile([128, BC], BF16, name=f"s1_{i}") for i in range(2)]
        sd0 = [const.tile([65, BC], BF16, name=f"sd0_{i}") for i in range(2)]
        sd1 = [const.tile([128, BC], BF16, name=f"sd1_{i}") for i in range(2)]

        nc.vector.memset(s0[0][LO, :], 0.0)
        nc.vector.memset(s1[0][:, :], 0.0)
        nc.vector.memset(s1[1][HI, :], 0.0)
        nc.sync.dma_start(out=s0[0][64:70, :], in_=dram['xT'][0])

        def hs(ap, h):
            return ap[:, h * M:(h + 1) * M]

        # ================= encoder =================
        eps = tc.alloc_tile_pool(name="eps", bufs=1, space="PSUM")
        for k in range(L + 1):
            S0, S0n = s0[k % 3], s0[(k + 1) % 3]
            S1, S1n = s1[k % 2], s1[(k + 1) % 2]
            if k + 1 < L:
                nc.sync.dma_start(out=S0n[64:70, :], in_=dram['xT'][k + 1])
            has0, has1 = k < L, k >= 1
            if has0 and has1:
                part = slice(0, 128)
            else:
                part = LO if has0 else HI

            for h in range(2):
                pzr = eps.tile([128, 2 * M], F32, name=f"pzr{h}_{k}",
                               tag=f"zr{h}")
                pin = eps.tile([128, M], F32, name=f"pin{h}_{k}", tag=f"i{h}")
                phn = eps.tile([128, M], F32, name=f"phn{h}_{k}", tag=f"n{h}")
                szr = work.tile([128, 2 * M], BF16, name=f"szr{h}_{k}",
                                tag=f"szr{h}")
                rhn = work.tile([128, M], BF16, name=f"rhn{h}_{k}",
                                tag=f"rhn{h}")
                n_t = work.tile([128, M], BF16, name=f"n{h}_{k}", tag=f"nn{h}")
                u_t = work.tile([128, M], BF16, name=f"u{h}_{k}", tag=f"u{h}")
                v_t = work.tile([128, M], BF16, name=f"v{h}_{k}", tag=f"v{h}")
                S0h, Sh = hs(S0, h), hs(S1, h)

                # gate matmuls: 64-col stationaries, cell0->cols 0:64,
                # cell1->cols 64:128 of each bank; pairs to disjoint
                # col-groups run concurrently on the PE array
                if has0:
                    nc.tensor.matmul(pzr[LO, 0:M], cw['wz0'], S0h,
                                     start=True, stop=True,
                                     tile_position=(0, 0))
                if has1:
                    nc.tensor.matmul(pzr[HI, 0:M], cw['wz1'], Sh,
                                     start=True, stop=True,
                                     tile_position=(0, 64))
                if has0:
                    nc.tensor.matmul(pzr[LO, M:2 * M], cw['wr0'], S0h,
                                     start=True, stop=True,
                                     tile_position=(0, 0))
                if has1:
                    nc.tensor.matmul(pzr[HI, M:2 * M], cw['wr1'], Sh,
                                     start=True, stop=True,
                                     tile_position=(0, 64))
                if has0:
                    nc.tensor.matmul(pin[LO, :], cw['win0'], S0h,
                                     start=True, stop=False,
                                     tile_position=(0, 0))
                if has1:
                    nc.tensor.matmul(pin[HI, :], cw['winhn1'][LO, :],
                                     Sh[LO, :], start=True, stop=False,
                                     tile_position=(0, 64))
                if has0:
                    nc.tensor.matmul(phn[LO, :], cw['whn0'], S0h[LO, :],
                                     start=True, stop=True,
                                     tile_position=(0, 0))
                if has1:
                    nc.tensor.matmul(phn[HI, :], cw['winhn1'][HI, :],
                                     Sh[HI, :], start=True, stop=True,
                                     tile_position=(64, 64))
                # elementwise chain (both cells at once)
                nc.scalar.activation(out=szr[part, 0:M], in_=pzr[part, 0:M],
                                     func=ACTF.Sigmoid,
                                     bias=bcol(0)[part, :], scale=1.0)
                nc.scalar.activation(out=szr[part, M:2 * M],
                                     in_=pzr[part, M:2 * M],
                                     func=ACTF.Sigmoid,
                                     bias=bcol(1)[part, :], scale=1.0)
                nc.vector.scalar_tensor_tensor(
                    out=rhn[part, :], in0=phn[part, :],
                    scalar=bcol(3)[part, :], in1=szr[part, M:2 * M],
                    op0=ALU.add, op1=ALU.mult)
                # identity accumulation, split into a col-concurrent pair
                if has0:
                    nc.tensor.matmul(pin[LO, :], cw['ident'][LO, LO],
                                     rhn[LO, :], start=False, stop=True,
                                     tile_position=(0, 0))
                if has1:
                    nc.tensor.matmul(pin[HI, :], cw['ident'][HI, HI],
                                     rhn[HI, :], start=False, stop=True,
                                     tile_position=(64, 64))
                nc.scalar.activation(out=n_t[part, :], in_=pin[part, :],
                                     func=ACTF.Tanh, bias=bcol(2)[part, :],
                                     scale=1.0)
                nc.vector.tensor_tensor(out=u_t[part, :], in0=Sh[part, :],
                                        in1=n_t[part, :], op=ALU.subtract)
                nc.vector.tensor_tensor(out=v_t[part, :],
                                        in0=szr[part, 0:M],
                                        in1=u_t[part, :], op=ALU.mult)
                nc.vector.tensor_tensor(out=hs(S1n, h)[part, :],
                                        in0=n_t[part, :],
                                        in1=v_t[part, :], op=ALU.add)
                if k < L - 1:
                    nc.vector.tensor_copy(hs(S0n, h)[LO, :],
                                          hs(S1n, h)[LO, :])

        # ================= transition =================
        nc.vector.tensor_copy(sd0[0][LO, :], s1[L % 2][LO, :])
        nc.vector.tensor_copy(sd1[0][HI, :], s1[(L + 1) % 2][HI, :])
        nc.vector.memset(sd0[0][64:65, :], 0.0)
        eps.release()

        # ================= decoder =================
        dps = tc.alloc_tile_pool(name="dps", bufs=1, space="PSUM")
        for t in range(T):
            D0, D0n = sd0[t % 2], sd0[(t + 1) % 2]
            D1, D1n = sd1[t % 2], sd1[(t + 1) % 2]
            for h in range(2):
                n_t = work.tile([128, M], BF16, name=f"nd{h}_{t}",
                                tag=f"nn{h}")
                u_t = work.tile([128, M], BF16, name=f"ud{h}_{t}",
                                tag=f"u{h}")
                v_t = work.tile([128, M], BF16, name=f"vd{h}_{t}",
                                tag=f"v{h}")
                rz0 = work.tile([128, M], BF16, name=f"rz0{h}_{t}",
                                tag=f"rz0{h}")
                rz1 = work.tile([128, M], BF16, name=f"rz1{h}_{t}",
                                tag=f"rz1{h}")
                rhn = work.tile([128, M], BF16, name=f"rhnd{h}_{t}",
                                tag=f"rhn{h}")
                cvsb = work.tile([66, M], BF16, name=f"cvsb{h}_{t}",
                                 tag=f"cvsb{h}")
                p1_0 = dps.tile([128, M], F32, name=f"p10{h}_{t}",
                                tag=f"d0a{h}")
                p2_0 = dps.tile([128, M], F32, name=f"p20{h}_{t}",
                                tag=f"d0b{h}")
                p1_1 = dps.tile([128, M], F32, name=f"p11{h}_{t}",
                                tag=f"d1a{h}")
                p2_1 = dps.tile([128, M], F32, name=f"p21{h}_{t}",
                                tag=f"d1b{h}")
                pon = dps.tile([65, M], F32, name=f"pon{h}_{t}",
                               tag=f"d0a{h}")
                pcv = dps.tile([65, M], F32, name=f"pcv{h}_{t}",
                               tag=f"d0b{h}")
                D0h, D1h = hs(D0, h), hs(D1, h)
                D0nh, D1nh = hs(D0n, h), hs(D1n, h)

                # dec1 EARLY matmuls: Whh part, reads h1_{t-1} from D1[HI]
                nc.tensor.matmul(p1_1, cw['w1_d1'][HI, :], D1h[HI, :],
                                 start=True, stop=False,
                                 tile_position=(64, 0))
                nc.tensor.matmul(p2_1, cw['w2_d1'][HI, :], D1h[HI, :],
                                 start=True, stop=False,
                                 tile_position=(64, 0))
                # ---- dec0
                nc.tensor.matmul(p1_0, cw['w1_d0'], D0h,
                                 start=True, stop=True, tile_position=(0, 0))
                nc.tensor.matmul(p2_0, cw['w2_d0'], D0h,
                                 start=True, stop=True, tile_position=(0, 0))
                nc.scalar.activation(out=rz0, in_=p1_0, func=ACTF.Sigmoid,
                                     bias=bcol(4), scale=1.0)
                nc.vector.scalar_tensor_tensor(
                    out=rhn[HI, :], in0=p2_0[HI, :],
                    scalar=bcol(5)[HI, :], in1=rz0[HI, :],
                    op0=ALU.add, op1=ALU.mult)
                nc.tensor.matmul(p2_0[LO, :], cw['identb'][HI, :],
                                 rhn[HI, :], start=False, stop=True,
                                 tile_position=(64, 0))
                nc.scalar.activation(out=n_t[LO, :], in_=p2_0[LO, :],
                                     func=ACTF.Tanh, bias=bcol(5)[LO, :],
                                     scale=1.0)
                nc.vector.tensor_tensor(out=u_t[LO, :], in0=D0h[LO, :],
                                        in1=n_t[LO, :], op=ALU.subtract)
                nc.vector.tensor_tensor(out=v_t[LO, :], in0=rz0[LO, :],
                                        in1=u_t[LO, :], op=ALU.mult)
                nc.vector.tensor_tensor(out=D0nh[LO, :], in0=n_t[LO, :],
                                        in1=v_t[LO, :], op=ALU.add)
                # dec1 LATE matmuls: Wih part, reads h0d_t from D0n[LO]
                nc.tensor.matmul(p1_1, cw['w1_d1'][LO, :], D0nh[LO, :],
                                 start=False, stop=True,
                                 tile_position=(0, 0))
                nc.tensor.matmul(p2_1, cw['w2_d1'][LO, :], D0nh[LO, :],
                                 start=False, stop=True,
                                 tile_position=(0, 0))
                nc.scalar.activation(out=rz1, in_=p1_1, func=ACTF.Sigmoid,
                                     bias=bcol(6), scale=1.0)
                nc.vector.scalar_tensor_tensor(
                    out=rhn[LO, :], in0=p2_1[LO, :],
                    scalar=bcol(7)[LO, :], in1=rz1[LO, :],
                    op0=ALU.add, op1=ALU.mult)
                nc.tensor.matmul(p2_1[HI, :], cw['identb'][LO, :],
                                 rhn[LO, :], start=False, stop=True,
                                 tile_position=(0, 64))
                nc.scalar.activation(out=n_t[HI, :], in_=p2_1[HI, :],
                                     func=ACTF.Tanh, bias=bcol(7)[HI, :],
                                     scale=1.0)
                nc.vector.tensor_tensor(out=u_t[HI, :], in0=D1h[HI, :],
                                        in1=n_t[HI, :], op=ALU.subtract)
                nc.vector.tensor_tensor(out=v_t[HI, :], in0=rz1[HI, :],
                                        in1=u_t[HI, :], op=ALU.mult)
                nc.vector.tensor_tensor(out=D1nh[HI, :], in0=n_t[HI, :],
                                        in1=v_t[HI, :], op=ALU.add)
                # heads on h1' (= D1n[HI])
                nc.tensor.matmul(pon[64:65, :], whead[64:128, 0:1],
                                 D1nh[HI, :], start=True, stop=True,
                                 tile_position=(64, 64))
                nc.tensor.matmul(pcv[64:65, :], whead[64:128, 1:2],
                                 D1nh[HI, :], start=True, stop=True,
                                 tile_position=(64, 64))
                nc.scalar.activation(out=cvsb[64:65, :], in_=pcv[64:65, :],
                                     func=ACTF.Identity,
                                     bias=bias[64:65, 9:10], scale=1.0)
                nc.vector.scalar_tensor_tensor(
                    out=D0nh[64:65, :], in0=pon[64:65, :],
                    scalar=bias[64:65, 8:9], in1=cvsb[64:65, :],
                    op0=ALU.is_gt, op1=ALU.mult)
            nc.gpsimd.dma_start(out=stg[t:t + 1, :], in_=D0n[64:65, :])

        dps.release()
        work.release()
        const.release()

    nc.compile()
    return nc


def _get_nc(L, T, BC):
    key = (L, T, BC)
    if key not in _BUILD_CACHE:
        _BUILD_CACHE[key] = _build(L, T, BC)
    return _BUILD_CACHE[key]


# ------------------------------------------------------------------ entry point
def kernel(**inputs):
    x = np.asarray(inputs['x'])
    B, L, _ = x.shape
    T = T_OUT
    BC = B // NCORES
    nc = _get_nc(L, T, BC)

    packed = _prep(inputs, BC)
    in_maps = []
    for c in range(NCORES):
        xs = x[c * BC:(c + 1) * BC].astype(np.float32)      # (BC, L, 6)
        xT = np.ascontiguousarray(xs.transpose(1, 2, 0)).astype(NPBF)
        m = dict(packed)
        m['xT'] = xT
        in_maps.append(m)

    res = run_bass_kernel_spmd(nc, in_maps, core_ids=list(range(NCORES)))
    out = np.empty((B, T, 1), np.float32)
    for c in range(NCORES):
        stg = np.asarray(res.results[c]['stg'], np.float32)  # (T, BC)
        out[c * BC:(c + 1) * BC, :, 0] = stg.T
    return out
